# revision 23
# baseline (speedup 1.0000x reference)
"""Trainium2 Bass kernel for the MFVI second-order CRF message passing.

fp8-DoubleRow design (vs fp32r shifted-conv baseline):
  * conv matmuls in e4m3 with MatmulPerfMode.DoubleRow: one instruction
    covers the full 256-deep contraction at 0.5 cycles/output-row (4x
    the fp32r rate); +-1/+-2 sequence shifts are free-dim byte offsets
    of the moving operand into a guard-padded [128, 2, S+4] tile.
  * the unary+corrections add is folded INTO each conv PSUM group as
    extra DoubleRow matmuls: identity-block weights x fp8(ucorr)
    moving operand (iter 0: single e4m3; iter 1: e4m3 + e4m3-residual
    pair for near-exact add, since its output feeds the final conv).
  * softmax chain per iteration: Act exp reads msg+ucorr straight from
    PSUM with bias=-ln(512) (e5m2 range guard) and writes e5m2 "t";
    colsum = e5m2 DoubleRow ones/16-matmul + a 1-partition epsilon
    matmul (masked positions give Z=eps, never 0 -> no inf/NaN);
    rb = 1/pz on DVE; conv input qsm8 = t*rb (= 16*qhat) in e4m3,
    chunk0 on DVE, chunk1 on Pool.
  * iteration 0 is host-side input prep: 16*softmax(unary) shipped as
    e4m3, so the device pipeline starts with a conv.
  * masked positions: ucorr = -240 -> exp underflows to exact 0; the
    reference's uniform-softmax messages from masked senders into
    len-1/len-2 are host-folded into the unary corrections.
  * final iteration: raw msg2 PSUM evacuated to bf16 (DVE st0 / Act
    st1), and (ucorr + msg2) * mask runs on the host in f32.
  * software pipeline: per step [conv | mid-conv colsum of previous
    step | exp/prep | zchain tail], one-step lag, deep SBUF buffer
    rotation; PSUM: 2 conv tags x 3 bufs x 1 bank + colsum 2 banks.
Data-parallel over batch: 4 elems per core, 8 cores, no collectives.
"""
import sys

sys.path.insert(0, "/opt/trn_rl_repo")

import math
import numpy as np
import ml_dtypes

import concourse.mybir as mybir
from concourse.bass import Bass
from concourse.tile import TileContext
from concourse import bass_utils

B, S, T = 32, 1024, 256
W = 2
ITERS = 3
N_CORES = 8
BPC = B // N_CORES
NCH = T // 128
HALF = S // 2
SP = S + 2 * W
K_EXP = 512.0
QSCALE = 16.0
LN_K = math.log(K_EXP)

f32 = mybir.dt.float32
bf16 = mybir.dt.bfloat16
e4 = mybir.dt.float8e4
e5 = mybir.dt.float8e5

SHIFT_D = (-1, -2, +1, +2)  # mats: T1, T2, T1^T, T2^T


def _split_sync_waits(nc):
    ctr = 0
    for f in nc.m.functions:
        for block in f.blocks:
            out = []
            changed = False
            for inst in block.instructions:
                si = inst.sync_info
                waits = list(si.on_wait) if si is not None and si.on_wait else []
                if len(waits) > 1:
                    changed = True
                    for w in waits[:-1]:
                        ctr += 1
                        nop = mybir.InstNoOp(
                            name=f"I-waitsplit-{ctr}",
                            engine=inst.engine, ins=[], outs=[])
                        nop.sync_info = mybir.SyncInfo(on_wait=[w], on_update=[])
                        out.append(nop)
                    si.on_wait = [waits[-1]]
                    inst.sync_info = si
                out.append(inst)
            if changed:
                block.instructions = out
    return nc


def _build():
    nc = Bass(trn_type="TRN2", target_bir_lowering=False, debug=False,
              num_devices=N_CORES)

    qs0_d = nc.dram_tensor("qs0", [BPC, 128, NCH, SP], e4,
                           kind="ExternalInput").ap()
    u8_d = nc.dram_tensor("u8", [BPC, 128, NCH, S], e4,
                          kind="ExternalInput").ap()
    w8_d = nc.dram_tensor("w8", [128, NCH * 5 * NCH, 128], e4,
                          kind="ExternalInput").ap()
    wz_d = nc.dram_tensor("wz", [128, NCH, 128], e5,
                          kind="ExternalInput").ap()
    wepse_d = nc.dram_tensor("wepse", [1, NCH, 128 + HALF], e5,
                             kind="ExternalInput").ap()
    qout = nc.dram_tensor("qout", [BPC, 128, NCH, S], bf16,
                          kind="ExternalOutput").ap()

    DR = mybir.MatmulPerfMode.DoubleRow

    with TileContext(nc) as tc:
        with tc.tile_pool(name="persist", bufs=1) as pp, \
             tc.tile_pool(name="work", bufs=2) as wp, \
             tc.tile_pool(name="psum", bufs=2, space="PSUM") as psp:

            qsA = [pp.tile([128, NCH, SP], e4, tag=f"qsa{n}", name=f"qsa{n}")
                   for n in range(BPC)]
            qsB = [pp.tile([128, NCH, SP], e4, tag=f"qsb{n}", name=f"qsb{n}")
                   for n in range(BPC)]
            w8all = pp.tile([128, NCH * 5 * NCH, 128], e4,
                            tag="w8", name="w8all")
            wzt = pp.tile([128, NCH, 128], e5, tag="wz", name="wzt")
            wepse = pp.tile([1, NCH, 128 + HALF], e5, tag="wepse",
                            name="wepse")
            wepst = wepse[:, :, 0:128]
            epst = wepse[:, :, 128:128 + HALF]
            u8t = [pp.tile([128, NCH, S], e4, tag=f"u8{n}", name=f"u8{n}")
                   for n in range(BPC)]

            # ramp-starter tensors first: one tiny matmul sets pe_busy_start
            # so real convs hit full p-state; no long warm-up needed.
            ones_f = pp.tile([128, 16], f32, tag="ones_f", name="ones_f")
            nc.vector.memset(ones_f[:], 1.0)
            ones_m = pp.tile([128, 16], mybir.dt.float32r, tag="ones_m",
                             name="ones_m")
            nc.vector.tensor_copy(out=ones_m[:], in_=ones_f[:])

            # input DMAs ordered for earliest first-conv start; u8[0] split so
            # its first half lands before the it0/n0 identity matmuls.
            nc.sync.dma_start(out=qsA[0], in_=qs0_d[0])
            nc.sync.dma_start(out=w8all, in_=w8_d)
            nc.sync.dma_start(out=u8t[0][:, :, 0:HALF], in_=u8_d[0][:, :, 0:HALF])
            nc.sync.dma_start(out=u8t[0][:, :, HALF:S], in_=u8_d[0][:, :, HALF:S])
            for n in range(1, BPC):
                nc.sync.dma_start(out=qsA[n], in_=qs0_d[n])
                nc.sync.dma_start(out=u8t[n], in_=u8_d[n])
            nc.sync.dma_start(out=wzt, in_=wz_d)
            nc.sync.dma_start(out=wepse, in_=wepse_d)

            bln = pp.tile([128, 1], f32, tag="bln", name="bln")
            nc.vector.memset(bln[:], -LN_K)

            zg = pp.tile([128, NCH, W], f32, tag="zg", name="zg")
            nc.vector.memset(zg[:], 0.0)
            for n in range(BPC):
                nc.vector.tensor_copy(out=qsB[n][:, :, 0:W], in_=zg[:])
                nc.vector.tensor_copy(out=qsB[n][:, :, S + W:SP], in_=zg[:])

            pwarm = psp.tile([128, S], f32, tag="z", name="pwarm", bufs=1)
            nc.tensor.matmul(pwarm[0:16, 0:16], ones_m[:], ones_m[:],
                             start=True, stop=True)

            def emit_conv_st(it, n, qs_ap, st, pmq):
                # shifts for both chunks first, then the ucorr adds: the
                # first step's shift inputs land before u8 does.
                for c in range(NCH):
                    pm = psp.tile([128, HALF], f32, tag=f"m{c}",
                                  name=f"pm_{it}_{n}_{st}_{c}", bufs=3)
                    pmq[(st, c)] = pm
                    for mi, d in enumerate(SHIFT_D):
                        lo = W + st * HALF + d
                        b0 = (c * 5 + mi) * NCH
                        nc.tensor.matmul(
                            pm[:], w8all[:, b0:b0 + 2, :],
                            qs_ap[:, :, lo:lo + HALF],
                            start=(mi == 0), stop=False,
                            perf_mode=DR)
                for c in range(NCH):
                    pm = pmq[(st, c)]
                    bi = (c * 5 + 4) * NCH
                    if it == 0:
                        # +ucorr via fp8 identity (noise damped by
                        # two subsequent softmax iterations)
                        nc.tensor.matmul(
                            pm[:], w8all[:, bi:bi + 2, :],
                            u8t[n][:, :, st * HALF:(st + 1) * HALF],
                            start=False, stop=True, perf_mode=DR)
                    else:
                        # +ucorr via single fp8 identity (the e4m3
                        # rounding noise stays within the rel-err budget)
                        nc.tensor.matmul(
                            pm[:], w8all[:, bi:bi + 2, :],
                            u8t[n][:, :, st * HALF:(st + 1) * HALF],
                            start=False, stop=True, perf_mode=DR)
                return pmq

            def emit_conv(it, n, qs_ap):
                pmq = {}
                emit_conv_st(it, n, qs_ap, 0, pmq)
                emit_conv_st(it, n, qs_ap, 1, pmq)
                return pmq

            def emit_conv_it2(n, qs_ap):
                # final-iteration conv: raw msg2 in PSUM; evac emitted
                # separately (after the ripe zchain tail) so critical
                # normalization legs get engine priority.
                pmq = {}
                for st in range(2):
                    for c in range(NCH):
                        pm = psp.tile([128, HALF], f32, tag=f"m{c}",
                                      name=f"pm_2_{n}_{st}_{c}", bufs=3)
                        pmq[(st, c)] = pm
                        for mi, d in enumerate(SHIFT_D):
                            lo = W + st * HALF + d
                            b0 = (c * 5 + mi) * NCH
                            nc.tensor.matmul(
                                pm[:], w8all[:, b0:b0 + 2, :],
                                qs_ap[:, :, lo:lo + HALF],
                                start=(mi == 0), stop=(mi == 3),
                                perf_mode=DR)
                return pmq

            def emit_evac_it2(n, pmq):
                # PSUM -> bf16 SBUF on Pool/Act/DVE, then one DMA per n
                # (two for the last n so its st0 half ships early and the
                # tail transfer is the st1 half only).
                evt = wp.tile([128, NCH, S], bf16, tag="ev",
                              name=f"ev_{n}", bufs=3)
                last = n == BPC - 1
                for st in range(2):
                    for c in range(NCH):
                        dst = evt[:, c, st * HALF:(st + 1) * HALF]
                        src = pmq[(st, c)][:]
                        if c == 1:
                            nc.scalar.copy(out=dst, in_=src)
                        elif st == 0 and n >= 2:
                            nc.gpsimd.tensor_copy(out=dst, in_=src)
                        else:
                            nc.vector.tensor_copy(out=dst, in_=src)
                    if last:
                        nc.sync.dma_start(
                            out=qout[n][:, :, st * HALF:(st + 1) * HALF],
                            in_=evt[:, :, st * HALF:(st + 1) * HALF])
                if not last:
                    nc.sync.dma_start(out=qout[n], in_=evt)

            def emit_prep(it_next, n, pmq):
                # t = exp(pm - ln512) from PSUM quadrants (Act), bf16
                e1 = wp.tile([128, NCH, S], e5, tag="e1",
                             name=f"e1_{it_next}_{n}", bufs=8)
                for st, c in ((0, 0), (0, 1), (1, 0), (1, 1)):
                    nc.scalar.activation(
                        out=e1[:, c, st * HALF:(st + 1) * HALF],
                        in_=pmq[(st, c)][:],
                        func=mybir.ActivationFunctionType.Exp,
                        bias=bln[:])
                return e1, e1

            def emit_zchain_colsum(n, qu):
                pz = psp.tile([128, S], f32, tag="z", name=f"z_{n}", bufs=1)
                for h in range(2):
                    nc.tensor.matmul(pz[:, h * HALF:(h + 1) * HALF],
                                     wepst[:], epst[:],
                                     start=True, stop=False, perf_mode=DR)
                    nc.tensor.matmul(pz[:, h * HALF:(h + 1) * HALF],
                                     wzt[:], qu[:, :, h * HALF:(h + 1) * HALF],
                                     start=False, stop=True, perf_mode=DR)
                return pz

            SPL = HALF + 2  # conv-st0 reads qs cols [0,516); split past it

            def emit_zchain_tail(n, e1, pz, qs_write):
                # split reciprocal per half so the consumer conv's st0 legs
                # start ~600ns earlier; c0 on DVE, c1 on Pool per half
                rb = wp.tile([128, S], f32, tag="rb", name=f"rb_{n}", bufs=8)
                nc.vector.reciprocal(rb[:, 0:SPL], pz[:, 0:SPL])
                nc.vector.tensor_mul(out=qs_write[:, 0, W:W + SPL],
                                     in0=e1[:, 0, 0:SPL], in1=rb[:, 0:SPL])
                nc.gpsimd.tensor_mul(out=qs_write[:, 1, W:W + SPL],
                                     in0=e1[:, 1, 0:SPL], in1=rb[:, 0:SPL])
                nc.vector.reciprocal(rb[:, SPL:S], pz[:, SPL:S])
                nc.vector.tensor_mul(out=qs_write[:, 0, W + SPL:W + S],
                                     in0=e1[:, 0, SPL:S], in1=rb[:, SPL:S])
                nc.gpsimd.tensor_mul(out=qs_write[:, 1, W + SPL:W + S],
                                     in0=e1[:, 1, SPL:S], in1=rb[:, SPL:S])

            STEPS = [(0, 0), (0, 1), (0, 2), (0, 3),
                     (1, 0), (1, 1), (1, 2), (1, 3),
                     (2, 0), (2, 1), (2, 2), (2, 3)]
            pending = []
            for it, n in STEPS:
                qs_read = qsA[n] if it != 1 else qsB[n]
                ripe = pending[0] if pending and pending[0][0] >= 1 else None
                if it < 2:
                    pmq = {}
                    emit_conv_st(it, n, qs_read, 0, pmq)
                    if ripe:
                        zn, e1p, qup, qsw = ripe[1]
                        ripe.append(emit_zchain_colsum(zn, qup))
                    emit_conv_st(it, n, qs_read, 1, pmq)
                    if ripe:
                        pending.pop(0)
                        zn, e1p, qup, qsw = ripe[1]
                        emit_zchain_tail(zn, e1p, ripe[2], qsw)
                        ripe = None
                    qs_write = qsB[n] if it == 0 else qsA[n]
                    e1, qu = emit_prep(it + 1, n, pmq)
                    pending.append([0, (n, e1, qu, qs_write)])
                else:
                    pmq = emit_conv_it2(n, qs_read)
                    if ripe:
                        zn, e1p, qup, qsw = ripe[1]
                        ripe.append(emit_zchain_colsum(zn, qup))
                    if ripe:
                        pending.pop(0)
                        zn, e1p, qup, qsw = ripe[1]
                        emit_zchain_tail(zn, e1p, ripe[2], qsw)
                    emit_evac_it2(n, pmq)
                for p in pending:
                    p[0] += 1
            while pending:
                p = pending.pop(0)
                pz = emit_zchain_colsum(p[1][0], p[1][2])
                emit_zchain_tail(p[1][0], p[1][1], pz, p[1][3])

    _split_sync_waits(nc)
    return nc


_CACHED_NC = None


def _get_nc():
    global _CACHED_NC
    if _CACHED_NC is None:
        _CACHED_NC = _build()
    return _CACHED_NC


def _host_prep(token_feats, unary_score, mask, transitions, start_transitions,
               end_transitions, lengths):
    mask = np.asarray(mask, np.float32)
    unary_score = np.asarray(unary_score, np.float32)
    transitions = np.asarray(transitions, np.float32)
    start_transitions = np.asarray(start_transitions, np.float32)
    end_transitions = np.asarray(end_transitions, np.float32)
    lengths = np.asarray(lengths).astype(np.int64)

    unary = unary_score * mask[..., None]

    ucorr = unary.copy()
    ucorr[:, 0:W, :] += start_transitions[None, :, :]
    trow = transitions.mean(axis=2)
    for b in range(B):
        L = int(lengths[b])
        for j in range(1, W + 1):
            ucorr[b, L - j] += end_transitions[j - 1]
        for m in (L, L + 1):
            if m >= S:
                continue
            for j in range(1, W + 1):
                r = m - j
                if 0 <= r < L:
                    ucorr[b, r] += trow[j - 1]

    # e4m3 ucorr; masked -> -240 (exp -> 0)
    ucm = np.where(mask[..., None] > 0, ucorr, -240.0).astype(np.float32)
    u8 = ucm.astype(ml_dtypes.float8_e4m3)

    em = np.exp(unary - unary.max(-1, keepdims=True))
    q0 = em / em.sum(-1, keepdims=True)
    q0 = (q0 * mask[..., None] * QSCALE).astype(np.float32)

    def to_dev(x, pad):  # [B,S,T] -> [B, 128, NCH, S(+2*pad)]
        t = x.transpose(0, 2, 1).reshape(B, NCH, 128, S)
        t = np.ascontiguousarray(t.transpose(0, 2, 1, 3))
        if pad:
            tp = np.zeros((B, 128, NCH, S + 2 * pad), t.dtype)
            tp[:, :, :, pad:pad + S] = t
            t = tp
        return t

    qs0 = to_dev(q0, W).astype(ml_dtypes.float8_e4m3)
    u8t = to_dev(u8.astype(np.float32), 0).astype(ml_dtypes.float8_e4m3)

    mats = [transitions[0], transitions[1],
            transitions[0].T, transitions[1].T]
    w8 = np.zeros((128, NCH * 5 * NCH, 128), np.float32)
    for c in range(NCH):
        for mi in range(4):
            for kc in range(NCH):
                w8[:, (c * 5 + mi) * NCH + kc, :] = \
                    mats[mi][kc * 128:(kc + 1) * 128,
                             c * 128:(c + 1) * 128] / QSCALE
        w8[:, (c * 5 + 4) * NCH + c, :] = np.eye(128)
    w8 = w8.astype(ml_dtypes.float8_e4m3)

    wz = np.full((128, NCH, 128), 1.0 / 16, ml_dtypes.float8_e5m2)
    wepse = np.zeros((1, NCH, 128 + HALF), ml_dtypes.float8_e5m2)
    wepse[:, :, 0:128] = np.float32(1.0 / 16)
    wepse[:, :, 128:] = np.float32(2.0 ** -14)

    return qs0, u8t, w8, wz, wepse, ucorr, mask


def kernel(token_feats, unary_score, mask, transitions, start_transitions,
           end_transitions, lengths):
    qs0, u8t, w8, wz, wepse, ucorr, maskf = _host_prep(
        token_feats, unary_score, mask, transitions, start_transitions,
        end_transitions, lengths)

    in_maps = []
    for core in range(N_CORES):
        sl = slice(core * BPC, (core + 1) * BPC)
        in_maps.append({
            "qs0": np.ascontiguousarray(qs0[sl]),
            "u8": np.ascontiguousarray(u8t[sl]),
            "w8": w8,
            "wz": wz,
            "wepse": wepse,
        })

    nc = _get_nc()
    res = bass_utils.run_bass_kernel_spmd(nc, in_maps,
                                          core_ids=list(range(N_CORES)))
    qT = np.concatenate(
        [np.asarray(res.results[c]["qout"], dtype=np.float32)
         for c in range(N_CORES)], axis=0)  # [B, 128, NCH, S]
    msg = qT.transpose(0, 3, 2, 1).reshape(B, S, T)
    out = (ucorr + msg) * maskf[..., None]
    return np.ascontiguousarray(out.astype(np.float32))



# revision 25
# speedup vs baseline: 1.0252x; 1.0252x over previous
"""Trainium2 Bass kernel for the MFVI second-order CRF message passing.

fp8-DoubleRow design (vs fp32r shifted-conv baseline):
  * conv matmuls in e4m3 with MatmulPerfMode.DoubleRow: one instruction
    covers the full 256-deep contraction at 0.5 cycles/output-row (4x
    the fp32r rate); +-1/+-2 sequence shifts are free-dim byte offsets
    of the moving operand into a guard-padded [128, 2, S+4] tile.
  * the unary+corrections add is folded INTO each conv PSUM group as
    extra DoubleRow matmuls: identity-block weights x fp8(ucorr)
    moving operand (iter 0: single e4m3; iter 1: e4m3 + e4m3-residual
    pair for near-exact add, since its output feeds the final conv).
  * softmax chain per iteration: Act exp reads msg+ucorr straight from
    PSUM with bias=-ln(512) (e5m2 range guard) and writes e5m2 "t";
    colsum = e5m2 DoubleRow ones/16-matmul + a 1-partition epsilon
    matmul (masked positions give Z=eps, never 0 -> no inf/NaN);
    rb = 1/pz on DVE; conv input qsm8 = t*rb (= 16*qhat) in e4m3,
    chunk0 on DVE, chunk1 on Pool.
  * iteration 0 is host-side input prep: 16*softmax(unary) shipped as
    e4m3, so the device pipeline starts with a conv.
  * masked positions: ucorr = -240 -> exp underflows to exact 0; the
    reference's uniform-softmax messages from masked senders into
    len-1/len-2 are host-folded into the unary corrections.
  * final iteration: raw msg2 PSUM evacuated to bf16 (DVE st0 / Act
    st1), and (ucorr + msg2) * mask runs on the host in f32.
  * software pipeline: per step [conv | mid-conv colsum of previous
    step | exp/prep | zchain tail], one-step lag, deep SBUF buffer
    rotation; PSUM: 2 conv tags x 3 bufs x 1 bank + colsum 2 banks.
Data-parallel over batch: 4 elems per core, 8 cores, no collectives.
"""
import sys

sys.path.insert(0, "/opt/trn_rl_repo")

import math
import numpy as np
import ml_dtypes

import concourse.mybir as mybir
from concourse.bass import Bass
from concourse.tile import TileContext
from concourse import bass_utils

B, S, T = 32, 1024, 256
W = 2
ITERS = 3
N_CORES = 8
BPC = B // N_CORES
NCH = T // 128
HALF = S // 2
SP = S + 2 * W
K_EXP = 512.0
QSCALE = 16.0
LN_K = math.log(K_EXP)

f32 = mybir.dt.float32
bf16 = mybir.dt.bfloat16
e4 = mybir.dt.float8e4
e5 = mybir.dt.float8e5

SHIFT_D = (-1, -2, +1, +2)  # mats: T1, T2, T1^T, T2^T


def _split_sync_waits(nc):
    ctr = 0
    for f in nc.m.functions:
        for block in f.blocks:
            out = []
            changed = False
            for inst in block.instructions:
                si = inst.sync_info
                waits = list(si.on_wait) if si is not None and si.on_wait else []
                if len(waits) > 1:
                    changed = True
                    for w in waits[:-1]:
                        ctr += 1
                        nop = mybir.InstNoOp(
                            name=f"I-waitsplit-{ctr}",
                            engine=inst.engine, ins=[], outs=[])
                        nop.sync_info = mybir.SyncInfo(on_wait=[w], on_update=[])
                        out.append(nop)
                    si.on_wait = [waits[-1]]
                    inst.sync_info = si
                out.append(inst)
            if changed:
                block.instructions = out
    return nc


def _build():
    nc = Bass(trn_type="TRN2", target_bir_lowering=False, debug=False,
              num_devices=N_CORES)

    qs0_d = nc.dram_tensor("qs0", [BPC, 128, NCH, SP], e4,
                           kind="ExternalInput").ap()
    u8_d = nc.dram_tensor("u8", [BPC, 128, NCH, S], e4,
                          kind="ExternalInput").ap()
    w8_d = nc.dram_tensor("w8", [128, NCH * 5 * NCH, 128], e4,
                          kind="ExternalInput").ap()
    wz_d = nc.dram_tensor("wz", [128, NCH, 128], e5,
                          kind="ExternalInput").ap()
    wepse_d = nc.dram_tensor("wepse", [1, NCH, 128 + HALF], e5,
                             kind="ExternalInput").ap()
    qout = nc.dram_tensor("qout", [BPC, 128, NCH, S], bf16,
                          kind="ExternalOutput").ap()

    DR = mybir.MatmulPerfMode.DoubleRow

    with TileContext(nc) as tc:
        with tc.tile_pool(name="persist", bufs=1) as pp, \
             tc.tile_pool(name="work", bufs=2) as wp, \
             tc.tile_pool(name="psum", bufs=2, space="PSUM") as psp:

            qsA = [pp.tile([128, NCH, SP], e4, tag=f"qsa{n}", name=f"qsa{n}")
                   for n in range(BPC)]
            qsB = [pp.tile([128, NCH, SP], e4, tag=f"qsb{n}", name=f"qsb{n}")
                   for n in range(BPC)]
            w8all = pp.tile([128, NCH * 5 * NCH, 128], e4,
                            tag="w8", name="w8all")
            wzt = pp.tile([128, NCH, 128], e5, tag="wz", name="wzt")
            wepse = pp.tile([1, NCH, 128 + HALF], e5, tag="wepse",
                            name="wepse")
            wepst = wepse[:, :, 0:128]
            epst = wepse[:, :, 128:128 + HALF]
            u8t = [pp.tile([128, NCH, S], e4, tag=f"u8{n}", name=f"u8{n}")
                   for n in range(BPC)]

            # ramp-starter tensors first: one tiny matmul sets pe_busy_start
            # so real convs hit full p-state; no long warm-up needed.
            ones_f = pp.tile([128, 16], f32, tag="ones_f", name="ones_f")
            nc.vector.memset(ones_f[:], 1.0)
            ones_m = pp.tile([128, 16], mybir.dt.float32r, tag="ones_m",
                             name="ones_m")
            nc.vector.tensor_copy(out=ones_m[:], in_=ones_f[:])

            # input DMAs ordered for earliest first-conv start; u8[0] split so
            # its first half lands before the it0/n0 identity matmuls.
            nc.sync.dma_start(out=qsA[0], in_=qs0_d[0])
            nc.sync.dma_start(out=w8all, in_=w8_d)
            nc.sync.dma_start(out=u8t[0][:, :, 0:HALF], in_=u8_d[0][:, :, 0:HALF])
            nc.sync.dma_start(out=u8t[0][:, :, HALF:S], in_=u8_d[0][:, :, HALF:S])
            for n in range(1, BPC):
                nc.sync.dma_start(out=qsA[n], in_=qs0_d[n])
                nc.sync.dma_start(out=u8t[n], in_=u8_d[n])
            nc.sync.dma_start(out=wzt, in_=wz_d)
            nc.sync.dma_start(out=wepse, in_=wepse_d)

            bln = pp.tile([128, 1], f32, tag="bln", name="bln")
            nc.vector.memset(bln[:], -LN_K)

            zg = pp.tile([128, NCH, W], f32, tag="zg", name="zg")
            nc.vector.memset(zg[:], 0.0)
            for n in range(BPC):
                nc.vector.tensor_copy(out=qsB[n][:, :, 0:W], in_=zg[:])
                nc.vector.tensor_copy(out=qsB[n][:, :, S + W:SP], in_=zg[:])

            pwarm = psp.tile([128, S], f32, tag="z", name="pwarm", bufs=1)
            nc.tensor.matmul(pwarm[0:16, 0:16], ones_m[:], ones_m[:],
                             start=True, stop=True)

            def emit_conv_st(it, n, qs_ap, st, pmq):
                # shifts for both chunks first, then the ucorr adds: the
                # first step's shift inputs land before u8 does.
                for c in range(NCH):
                    pm = psp.tile([128, HALF], f32, tag=f"m{c}",
                                  name=f"pm_{it}_{n}_{st}_{c}", bufs=3)
                    pmq[(st, c)] = pm
                    for mi, d in enumerate(SHIFT_D):
                        lo = W + st * HALF + d
                        b0 = (c * 5 + mi) * NCH
                        nc.tensor.matmul(
                            pm[:], w8all[:, b0:b0 + 2, :],
                            qs_ap[:, :, lo:lo + HALF],
                            start=(mi == 0), stop=False,
                            perf_mode=DR)
                for c in range(NCH):
                    pm = pmq[(st, c)]
                    bi = (c * 5 + 4) * NCH
                    if it == 0:
                        # +ucorr via fp8 identity (noise damped by
                        # two subsequent softmax iterations)
                        nc.tensor.matmul(
                            pm[:], w8all[:, bi:bi + 2, :],
                            u8t[n][:, :, st * HALF:(st + 1) * HALF],
                            start=False, stop=True, perf_mode=DR)
                    else:
                        # +ucorr via single fp8 identity (the e4m3
                        # rounding noise stays within the rel-err budget)
                        nc.tensor.matmul(
                            pm[:], w8all[:, bi:bi + 2, :],
                            u8t[n][:, :, st * HALF:(st + 1) * HALF],
                            start=False, stop=True, perf_mode=DR)
                return pmq

            def emit_conv(it, n, qs_ap):
                pmq = {}
                emit_conv_st(it, n, qs_ap, 0, pmq)
                emit_conv_st(it, n, qs_ap, 1, pmq)
                return pmq

            def emit_conv_it2(n, qs_ap):
                # final-iteration conv: raw msg2 in PSUM; evac emitted
                # separately (after the ripe zchain tail) so critical
                # normalization legs get engine priority.
                pmq = {}
                for st in range(2):
                    for c in range(NCH):
                        pm = psp.tile([128, HALF], f32, tag=f"m{c}",
                                      name=f"pm_2_{n}_{st}_{c}", bufs=3)
                        pmq[(st, c)] = pm
                        for mi, d in enumerate(SHIFT_D):
                            lo = W + st * HALF + d
                            b0 = (c * 5 + mi) * NCH
                            nc.tensor.matmul(
                                pm[:], w8all[:, b0:b0 + 2, :],
                                qs_ap[:, :, lo:lo + HALF],
                                start=(mi == 0), stop=(mi == 3),
                                perf_mode=DR)
                return pmq

            def emit_evac_it2(n, pmq):
                # PSUM -> bf16 SBUF: Act takes most halves (its exps are
                # done by mid-it2); Pool/DVE only take halves that become
                # ready after their own zchain legs have finished, so the
                # ready-first engine queues can't bypass-steal leg slots.
                evt = wp.tile([128, NCH, S], bf16, tag="ev",
                              name=f"ev_{n}", bufs=3)
                last = n == BPC - 1
                for st in range(2):
                    for c in range(NCH):
                        dst = evt[:, c, st * HALF:(st + 1) * HALF]
                        src = pmq[(st, c)][:]
                        if n == 3 and st == 1:
                            eng = (nc.vector.tensor_copy if c == 0
                                   else nc.gpsimd.tensor_copy)
                            eng(out=dst, in_=src)
                        elif n == 3 and st == 0 and c == 0:
                            nc.gpsimd.tensor_copy(out=dst, in_=src)
                        else:
                            nc.scalar.copy(out=dst, in_=src)
                    if last:
                        nc.sync.dma_start(
                            out=qout[n][:, :, st * HALF:(st + 1) * HALF],
                            in_=evt[:, :, st * HALF:(st + 1) * HALF])
                if not last:
                    nc.sync.dma_start(out=qout[n], in_=evt)

            def emit_prep(it_next, n, pmq):
                # t = exp(pm - ln512) from PSUM quadrants (Act), bf16
                e1 = wp.tile([128, NCH, S], e5, tag="e1",
                             name=f"e1_{it_next}_{n}", bufs=8)
                for st, c in ((0, 0), (0, 1), (1, 0), (1, 1)):
                    nc.scalar.activation(
                        out=e1[:, c, st * HALF:(st + 1) * HALF],
                        in_=pmq[(st, c)][:],
                        func=mybir.ActivationFunctionType.Exp,
                        bias=bln[:])
                return e1, e1

            def emit_zchain_colsum(n, qu):
                pz = psp.tile([128, S], f32, tag="z", name=f"z_{n}", bufs=1)
                for h in range(2):
                    nc.tensor.matmul(pz[:, h * HALF:(h + 1) * HALF],
                                     wepst[:], epst[:],
                                     start=True, stop=False, perf_mode=DR)
                    nc.tensor.matmul(pz[:, h * HALF:(h + 1) * HALF],
                                     wzt[:], qu[:, :, h * HALF:(h + 1) * HALF],
                                     start=False, stop=True, perf_mode=DR)
                return pz

            SPL = HALF + 2  # conv-st0 reads qs cols [0,516); split past it

            def emit_zchain_tail(n, e1, pz, qs_write):
                # split reciprocal per half so the consumer conv's st0 legs
                # start ~600ns earlier; per half: Pool takes most of c1,
                # DVE takes c0 plus the head of c1 so both legs finish
                # together (DVE 1.04 ns/elem vs Pool 2.16).
                CS = 180  # c1 head elems on DVE per half
                rb = wp.tile([128, S], f32, tag="rb", name=f"rb_{n}", bufs=8)
                nc.vector.reciprocal(rb[:, 0:SPL], pz[:, 0:SPL])
                nc.gpsimd.tensor_mul(out=qs_write[:, 1, W + CS:W + SPL],
                                     in0=e1[:, 1, CS:SPL], in1=rb[:, CS:SPL])
                nc.vector.tensor_mul(out=qs_write[:, 0, W:W + SPL],
                                     in0=e1[:, 0, 0:SPL], in1=rb[:, 0:SPL])
                nc.vector.tensor_mul(out=qs_write[:, 1, W:W + CS],
                                     in0=e1[:, 1, 0:CS], in1=rb[:, 0:CS])
                nc.vector.reciprocal(rb[:, SPL:S], pz[:, SPL:S])
                nc.gpsimd.tensor_mul(out=qs_write[:, 1, W + SPL + CS:W + S],
                                     in0=e1[:, 1, SPL + CS:S],
                                     in1=rb[:, SPL + CS:S])
                nc.vector.tensor_mul(out=qs_write[:, 0, W + SPL:W + S],
                                     in0=e1[:, 0, SPL:S], in1=rb[:, SPL:S])
                nc.vector.tensor_mul(out=qs_write[:, 1, W + SPL:W + SPL + CS],
                                     in0=e1[:, 1, SPL:SPL + CS],
                                     in1=rb[:, SPL:SPL + CS])

            STEPS = [(0, 0), (0, 1), (0, 2), (0, 3),
                     (1, 0), (1, 1), (1, 2), (1, 3),
                     (2, 0), (2, 1), (2, 2), (2, 3)]
            pending = []
            for it, n in STEPS:
                qs_read = qsA[n] if it != 1 else qsB[n]
                ripe = pending[0] if pending and pending[0][0] >= 1 else None
                if it < 2:
                    pmq = {}
                    emit_conv_st(it, n, qs_read, 0, pmq)
                    if ripe:
                        zn, e1p, qup, qsw = ripe[1]
                        ripe.append(emit_zchain_colsum(zn, qup))
                    emit_conv_st(it, n, qs_read, 1, pmq)
                    if ripe:
                        pending.pop(0)
                        zn, e1p, qup, qsw = ripe[1]
                        emit_zchain_tail(zn, e1p, ripe[2], qsw)
                        ripe = None
                    qs_write = qsB[n] if it == 0 else qsA[n]
                    e1, qu = emit_prep(it + 1, n, pmq)
                    pending.append([0, (n, e1, qu, qs_write)])
                else:
                    pmq = emit_conv_it2(n, qs_read)
                    if ripe:
                        zn, e1p, qup, qsw = ripe[1]
                        ripe.append(emit_zchain_colsum(zn, qup))
                    if ripe:
                        pending.pop(0)
                        zn, e1p, qup, qsw = ripe[1]
                        emit_zchain_tail(zn, e1p, ripe[2], qsw)
                    emit_evac_it2(n, pmq)
                for p in pending:
                    p[0] += 1
            while pending:
                p = pending.pop(0)
                pz = emit_zchain_colsum(p[1][0], p[1][2])
                emit_zchain_tail(p[1][0], p[1][1], pz, p[1][3])

    _split_sync_waits(nc)
    return nc


_CACHED_NC = None


def _get_nc():
    global _CACHED_NC
    if _CACHED_NC is None:
        _CACHED_NC = _build()
    return _CACHED_NC


def _host_prep(token_feats, unary_score, mask, transitions, start_transitions,
               end_transitions, lengths):
    mask = np.asarray(mask, np.float32)
    unary_score = np.asarray(unary_score, np.float32)
    transitions = np.asarray(transitions, np.float32)
    start_transitions = np.asarray(start_transitions, np.float32)
    end_transitions = np.asarray(end_transitions, np.float32)
    lengths = np.asarray(lengths).astype(np.int64)

    unary = unary_score * mask[..., None]

    ucorr = unary.copy()
    ucorr[:, 0:W, :] += start_transitions[None, :, :]
    trow = transitions.mean(axis=2)
    for b in range(B):
        L = int(lengths[b])
        for j in range(1, W + 1):
            ucorr[b, L - j] += end_transitions[j - 1]
        for m in (L, L + 1):
            if m >= S:
                continue
            for j in range(1, W + 1):
                r = m - j
                if 0 <= r < L:
                    ucorr[b, r] += trow[j - 1]

    # e4m3 ucorr; masked -> -240 (exp -> 0)
    ucm = np.where(mask[..., None] > 0, ucorr, -240.0).astype(np.float32)
    u8 = ucm.astype(ml_dtypes.float8_e4m3)

    em = np.exp(unary - unary.max(-1, keepdims=True))
    q0 = em / em.sum(-1, keepdims=True)
    q0 = (q0 * mask[..., None] * QSCALE).astype(np.float32)

    def to_dev(x, pad):  # [B,S,T] -> [B, 128, NCH, S(+2*pad)]
        t = x.transpose(0, 2, 1).reshape(B, NCH, 128, S)
        t = np.ascontiguousarray(t.transpose(0, 2, 1, 3))
        if pad:
            tp = np.zeros((B, 128, NCH, S + 2 * pad), t.dtype)
            tp[:, :, :, pad:pad + S] = t
            t = tp
        return t

    qs0 = to_dev(q0, W).astype(ml_dtypes.float8_e4m3)
    u8t = to_dev(u8.astype(np.float32), 0).astype(ml_dtypes.float8_e4m3)

    mats = [transitions[0], transitions[1],
            transitions[0].T, transitions[1].T]
    w8 = np.zeros((128, NCH * 5 * NCH, 128), np.float32)
    for c in range(NCH):
        for mi in range(4):
            for kc in range(NCH):
                w8[:, (c * 5 + mi) * NCH + kc, :] = \
                    mats[mi][kc * 128:(kc + 1) * 128,
                             c * 128:(c + 1) * 128] / QSCALE
        w8[:, (c * 5 + 4) * NCH + c, :] = np.eye(128)
    w8 = w8.astype(ml_dtypes.float8_e4m3)

    wz = np.full((128, NCH, 128), 1.0 / 16, ml_dtypes.float8_e5m2)
    wepse = np.zeros((1, NCH, 128 + HALF), ml_dtypes.float8_e5m2)
    wepse[:, :, 0:128] = np.float32(1.0 / 16)
    wepse[:, :, 128:] = np.float32(2.0 ** -14)

    return qs0, u8t, w8, wz, wepse, ucorr, mask


def kernel(token_feats, unary_score, mask, transitions, start_transitions,
           end_transitions, lengths):
    qs0, u8t, w8, wz, wepse, ucorr, maskf = _host_prep(
        token_feats, unary_score, mask, transitions, start_transitions,
        end_transitions, lengths)

    in_maps = []
    for core in range(N_CORES):
        sl = slice(core * BPC, (core + 1) * BPC)
        in_maps.append({
            "qs0": np.ascontiguousarray(qs0[sl]),
            "u8": np.ascontiguousarray(u8t[sl]),
            "w8": w8,
            "wz": wz,
            "wepse": wepse,
        })

    nc = _get_nc()
    res = bass_utils.run_bass_kernel_spmd(nc, in_maps,
                                          core_ids=list(range(N_CORES)))
    qT = np.concatenate(
        [np.asarray(res.results[c]["qout"], dtype=np.float32)
         for c in range(N_CORES)], axis=0)  # [B, 128, NCH, S]
    msg = qT.transpose(0, 3, 2, 1).reshape(B, S, T)
    out = (ucorr + msg) * maskf[..., None]
    return np.ascontiguousarray(out.astype(np.float32))



# revision 26
# speedup vs baseline: 1.0388x; 1.0133x over previous
"""Trainium2 Bass kernel for the MFVI second-order CRF message passing.

fp8-DoubleRow design (vs fp32r shifted-conv baseline):
  * conv matmuls in e4m3 with MatmulPerfMode.DoubleRow: one instruction
    covers the full 256-deep contraction at 0.5 cycles/output-row (4x
    the fp32r rate); +-1/+-2 sequence shifts are free-dim byte offsets
    of the moving operand into a guard-padded [128, 2, S+4] tile.
  * the unary+corrections add is folded INTO each conv PSUM group as
    extra DoubleRow matmuls: identity-block weights x fp8(ucorr)
    moving operand (iter 0: single e4m3; iter 1: e4m3 + e4m3-residual
    pair for near-exact add, since its output feeds the final conv).
  * softmax chain per iteration: Act exp reads msg+ucorr straight from
    PSUM with bias=-ln(512) (e5m2 range guard) and writes e5m2 "t";
    colsum = e5m2 DoubleRow ones/16-matmul + a 1-partition epsilon
    matmul (masked positions give Z=eps, never 0 -> no inf/NaN);
    rb = 1/pz on DVE; conv input qsm8 = t*rb (= 16*qhat) in e4m3,
    chunk0 on DVE, chunk1 on Pool.
  * iteration 0 is host-side input prep: 16*softmax(unary) shipped as
    e4m3, so the device pipeline starts with a conv.
  * masked positions: ucorr = -240 -> exp underflows to exact 0; the
    reference's uniform-softmax messages from masked senders into
    len-1/len-2 are host-folded into the unary corrections.
  * final iteration: raw msg2 PSUM evacuated to bf16 (DVE st0 / Act
    st1), and (ucorr + msg2) * mask runs on the host in f32.
  * software pipeline: per step [conv | mid-conv colsum of previous
    step | exp/prep | zchain tail], one-step lag, deep SBUF buffer
    rotation; PSUM: 2 conv tags x 3 bufs x 1 bank + colsum 2 banks.
Data-parallel over batch: 4 elems per core, 8 cores, no collectives.
"""
import sys

sys.path.insert(0, "/opt/trn_rl_repo")

import math
import numpy as np
import ml_dtypes

import concourse.mybir as mybir
from concourse.bass import Bass
from concourse.tile import TileContext
from concourse import bass_utils

B, S, T = 32, 1024, 256
W = 2
ITERS = 3
N_CORES = 8
BPC = B // N_CORES
NCH = T // 128
HALF = S // 2
SP = S + 2 * W
K_EXP = 512.0
QSCALE = 16.0
LN_K = math.log(K_EXP)

f32 = mybir.dt.float32
bf16 = mybir.dt.bfloat16
e4 = mybir.dt.float8e4
e5 = mybir.dt.float8e5

SHIFT_D = (-1, -2, +1, +2)  # mats: T1, T2, T1^T, T2^T


def _split_sync_waits(nc):
    ctr = 0
    for f in nc.m.functions:
        for block in f.blocks:
            out = []
            changed = False
            for inst in block.instructions:
                si = inst.sync_info
                waits = list(si.on_wait) if si is not None and si.on_wait else []
                if len(waits) > 1:
                    changed = True
                    for w in waits[:-1]:
                        ctr += 1
                        nop = mybir.InstNoOp(
                            name=f"I-waitsplit-{ctr}",
                            engine=inst.engine, ins=[], outs=[])
                        nop.sync_info = mybir.SyncInfo(on_wait=[w], on_update=[])
                        out.append(nop)
                    si.on_wait = [waits[-1]]
                    inst.sync_info = si
                out.append(inst)
            if changed:
                block.instructions = out
    return nc


def _build():
    nc = Bass(trn_type="TRN2", target_bir_lowering=False, debug=False,
              num_devices=N_CORES)

    qs0_d = nc.dram_tensor("qs0", [BPC, 128, NCH, SP], e4,
                           kind="ExternalInput").ap()
    u8_d = nc.dram_tensor("u8", [BPC, 128, NCH, S], e4,
                          kind="ExternalInput").ap()
    w8_d = nc.dram_tensor("w8", [128, NCH * 5 * NCH, 128], e4,
                          kind="ExternalInput").ap()
    wz_d = nc.dram_tensor("wz", [128, NCH, 128], e5,
                          kind="ExternalInput").ap()
    wepse_d = nc.dram_tensor("wepse", [1, NCH, 128 + HALF], e5,
                             kind="ExternalInput").ap()
    qout = nc.dram_tensor("qout", [BPC, 128, NCH, S], bf16,
                          kind="ExternalOutput").ap()

    DR = mybir.MatmulPerfMode.DoubleRow

    with TileContext(nc) as tc:
        with tc.tile_pool(name="persist", bufs=1) as pp, \
             tc.tile_pool(name="work", bufs=2) as wp, \
             tc.tile_pool(name="psum", bufs=2, space="PSUM") as psp:

            qsA = [pp.tile([128, NCH, SP], e4, tag=f"qsa{n}", name=f"qsa{n}")
                   for n in range(BPC)]
            qsB = [pp.tile([128, NCH, SP], e4, tag=f"qsb{n}", name=f"qsb{n}")
                   for n in range(BPC)]
            w8all = pp.tile([128, NCH * 5 * NCH, 128], e4,
                            tag="w8", name="w8all")
            wzt = pp.tile([128, NCH, 128], e5, tag="wz", name="wzt")
            wepse = pp.tile([1, NCH, 128 + HALF], e5, tag="wepse",
                            name="wepse")
            wepst = wepse[:, :, 0:128]
            epst = wepse[:, :, 128:128 + HALF]
            u8t = [pp.tile([128, NCH, S], e4, tag=f"u8{n}", name=f"u8{n}")
                   for n in range(BPC)]

            # ramp-starter tensors first: one tiny matmul sets pe_busy_start
            # so real convs hit full p-state; no long warm-up needed.
            ones_f = pp.tile([128, 16], f32, tag="ones_f", name="ones_f")
            nc.vector.memset(ones_f[:], 1.0)
            ones_m = pp.tile([128, 16], mybir.dt.float32r, tag="ones_m",
                             name="ones_m")
            nc.vector.tensor_copy(out=ones_m[:], in_=ones_f[:])

            # input DMAs ordered for earliest first-conv start; u8[0] split so
            # its first half lands before the it0/n0 identity matmuls.
            nc.sync.dma_start(out=qsA[0], in_=qs0_d[0])
            nc.sync.dma_start(out=w8all, in_=w8_d)
            nc.sync.dma_start(out=u8t[0][:, :, 0:HALF], in_=u8_d[0][:, :, 0:HALF])
            nc.sync.dma_start(out=u8t[0][:, :, HALF:S], in_=u8_d[0][:, :, HALF:S])
            for n in range(1, BPC):
                nc.sync.dma_start(out=qsA[n], in_=qs0_d[n])
                nc.sync.dma_start(out=u8t[n], in_=u8_d[n])
            nc.sync.dma_start(out=wzt, in_=wz_d)
            nc.sync.dma_start(out=wepse, in_=wepse_d)

            bln = pp.tile([128, 1], f32, tag="bln", name="bln")
            nc.vector.memset(bln[:], -LN_K)

            zg = pp.tile([128, NCH, W], f32, tag="zg", name="zg")
            nc.vector.memset(zg[:], 0.0)
            for n in range(BPC):
                nc.vector.tensor_copy(out=qsB[n][:, :, 0:W], in_=zg[:])
                nc.vector.tensor_copy(out=qsB[n][:, :, S + W:SP], in_=zg[:])

            pwarm = psp.tile([128, S], f32, tag="z", name="pwarm", bufs=1)
            nc.tensor.matmul(pwarm[0:16, 0:16], ones_m[:], ones_m[:],
                             start=True, stop=True)

            def emit_conv_st(it, n, qs_ap, st, pmq):
                # shifts for both chunks first, then the ucorr adds: the
                # first step's shift inputs land before u8 does.
                for c in range(NCH):
                    pm = psp.tile([128, HALF], f32, tag=f"m{c}",
                                  name=f"pm_{it}_{n}_{st}_{c}", bufs=3)
                    pmq[(st, c)] = pm
                    for mi, d in enumerate(SHIFT_D):
                        lo = W + st * HALF + d
                        b0 = (c * 5 + mi) * NCH
                        nc.tensor.matmul(
                            pm[:], w8all[:, b0:b0 + 2, :],
                            qs_ap[:, :, lo:lo + HALF],
                            start=(mi == 0), stop=False,
                            perf_mode=DR)
                for c in range(NCH):
                    pm = pmq[(st, c)]
                    bi = (c * 5 + 4) * NCH
                    if it == 0:
                        # +ucorr via fp8 identity (noise damped by
                        # two subsequent softmax iterations)
                        nc.tensor.matmul(
                            pm[:], w8all[:, bi:bi + 2, :],
                            u8t[n][:, :, st * HALF:(st + 1) * HALF],
                            start=False, stop=True, perf_mode=DR)
                    else:
                        # +ucorr via single fp8 identity (the e4m3
                        # rounding noise stays within the rel-err budget)
                        nc.tensor.matmul(
                            pm[:], w8all[:, bi:bi + 2, :],
                            u8t[n][:, :, st * HALF:(st + 1) * HALF],
                            start=False, stop=True, perf_mode=DR)
                return pmq

            def emit_conv(it, n, qs_ap):
                pmq = {}
                emit_conv_st(it, n, qs_ap, 0, pmq)
                emit_conv_st(it, n, qs_ap, 1, pmq)
                return pmq

            def emit_conv_it2(n, qs_ap):
                # final-iteration conv: raw msg2 in PSUM; evac emitted
                # separately (after the ripe zchain tail) so critical
                # normalization legs get engine priority.
                pmq = {}
                for st in range(2):
                    for c in range(NCH):
                        pm = psp.tile([128, HALF], f32, tag=f"m{c}",
                                      name=f"pm_2_{n}_{st}_{c}", bufs=3)
                        pmq[(st, c)] = pm
                        for mi, d in enumerate(SHIFT_D):
                            lo = W + st * HALF + d
                            b0 = (c * 5 + mi) * NCH
                            nc.tensor.matmul(
                                pm[:], w8all[:, b0:b0 + 2, :],
                                qs_ap[:, :, lo:lo + HALF],
                                start=(mi == 0), stop=(mi == 3),
                                perf_mode=DR)
                return pmq

            def emit_evac_it2(n, pmq):
                # PSUM -> bf16 SBUF: Act takes most halves (its exps are
                # done by mid-it2); Pool/DVE only take halves that become
                # ready after their own zchain legs have finished, so the
                # ready-first engine queues can't bypass-steal leg slots.
                evt = wp.tile([128, NCH, S], bf16, tag="ev",
                              name=f"ev_{n}", bufs=3)
                last = n == BPC - 1
                for st in range(2):
                    for c in range(NCH):
                        dst = evt[:, c, st * HALF:(st + 1) * HALF]
                        src = pmq[(st, c)][:]
                        if n == 3 and st == 1:
                            eng = (nc.vector.tensor_copy if c == 0
                                   else nc.gpsimd.tensor_copy)
                            eng(out=dst, in_=src)
                        elif n == 3 and st == 0 and c == 0:
                            nc.gpsimd.tensor_copy(out=dst, in_=src)
                        else:
                            nc.scalar.copy(out=dst, in_=src)
                    if last:
                        nc.sync.dma_start(
                            out=qout[n][:, :, st * HALF:(st + 1) * HALF],
                            in_=evt[:, :, st * HALF:(st + 1) * HALF])
                if not last:
                    nc.sync.dma_start(out=qout[n], in_=evt)

            def emit_prep(it_next, n, pmq):
                # t = exp(pm - ln512) from PSUM quadrants (Act), bf16
                e1 = wp.tile([128, NCH, S], e5, tag="e1",
                             name=f"e1_{it_next}_{n}", bufs=8)
                for st, c in ((0, 0), (0, 1), (1, 0), (1, 1)):
                    nc.scalar.activation(
                        out=e1[:, c, st * HALF:(st + 1) * HALF],
                        in_=pmq[(st, c)][:],
                        func=mybir.ActivationFunctionType.Exp,
                        bias=bln[:])
                return e1, e1

            def emit_zchain_colsum(n, qu):
                pz = psp.tile([128, S], f32, tag="z", name=f"z_{n}", bufs=1)
                for h in range(2):
                    nc.tensor.matmul(pz[:, h * HALF:(h + 1) * HALF],
                                     wepst[:], epst[:],
                                     start=True, stop=False, perf_mode=DR)
                    nc.tensor.matmul(pz[:, h * HALF:(h + 1) * HALF],
                                     wzt[:], qu[:, :, h * HALF:(h + 1) * HALF],
                                     start=False, stop=True, perf_mode=DR)
                return pz

            SPL = HALF + 2  # conv-st0 reads qs cols [0,516); split past it

            def emit_zchain_tail(n, e1, pz, qs_write):
                # split reciprocal per half so the consumer conv's st0 legs
                # start ~600ns earlier; per half: Pool takes most of c1,
                # DVE takes c0 plus the head of c1 so both legs finish
                # together (DVE 1.04 ns/elem vs Pool 2.16).
                rb = wp.tile([128, S], f32, tag="rb", name=f"rb_{n}", bufs=8)
                nc.vector.reciprocal(rb[:, 0:SPL], pz[:, 0:SPL])
                nc.gpsimd.tensor_mul(out=qs_write[:, 1, W:W + SPL],
                                     in0=e1[:, 1, 0:SPL], in1=rb[:, 0:SPL])
                nc.vector.tensor_mul(out=qs_write[:, 0, W:W + SPL],
                                     in0=e1[:, 0, 0:SPL], in1=rb[:, 0:SPL])
                nc.vector.reciprocal(rb[:, SPL:S], pz[:, SPL:S])
                nc.gpsimd.tensor_mul(out=qs_write[:, 1, W + SPL:W + S],
                                     in0=e1[:, 1, SPL:S], in1=rb[:, SPL:S])
                nc.vector.tensor_mul(out=qs_write[:, 0, W + SPL:W + S],
                                     in0=e1[:, 0, SPL:S], in1=rb[:, SPL:S])

            STEPS = [(0, 0), (0, 1), (0, 2), (0, 3),
                     (1, 0), (1, 1), (1, 2), (1, 3),
                     (2, 0), (2, 1), (2, 2), (2, 3)]
            pending = []
            for it, n in STEPS:
                qs_read = qsA[n] if it != 1 else qsB[n]
                ripe = pending[0] if pending and pending[0][0] >= 1 else None
                if it < 2:
                    pmq = {}
                    emit_conv_st(it, n, qs_read, 0, pmq)
                    if ripe:
                        zn, e1p, qup, qsw = ripe[1]
                        ripe.append(emit_zchain_colsum(zn, qup))
                    emit_conv_st(it, n, qs_read, 1, pmq)
                    if ripe:
                        pending.pop(0)
                        zn, e1p, qup, qsw = ripe[1]
                        emit_zchain_tail(zn, e1p, ripe[2], qsw)
                        ripe = None
                    qs_write = qsB[n] if it == 0 else qsA[n]
                    e1, qu = emit_prep(it + 1, n, pmq)
                    pending.append([0, (n, e1, qu, qs_write)])
                else:
                    pmq = emit_conv_it2(n, qs_read)
                    if ripe:
                        zn, e1p, qup, qsw = ripe[1]
                        ripe.append(emit_zchain_colsum(zn, qup))
                    if ripe:
                        pending.pop(0)
                        zn, e1p, qup, qsw = ripe[1]
                        emit_zchain_tail(zn, e1p, ripe[2], qsw)
                    emit_evac_it2(n, pmq)
                for p in pending:
                    p[0] += 1
            while pending:
                p = pending.pop(0)
                pz = emit_zchain_colsum(p[1][0], p[1][2])
                emit_zchain_tail(p[1][0], p[1][1], pz, p[1][3])

    _split_sync_waits(nc)
    return nc


_CACHED_NC = None


def _get_nc():
    global _CACHED_NC
    if _CACHED_NC is None:
        _CACHED_NC = _build()
    return _CACHED_NC


def _host_prep(token_feats, unary_score, mask, transitions, start_transitions,
               end_transitions, lengths):
    mask = np.asarray(mask, np.float32)
    unary_score = np.asarray(unary_score, np.float32)
    transitions = np.asarray(transitions, np.float32)
    start_transitions = np.asarray(start_transitions, np.float32)
    end_transitions = np.asarray(end_transitions, np.float32)
    lengths = np.asarray(lengths).astype(np.int64)

    unary = unary_score * mask[..., None]

    ucorr = unary.copy()
    ucorr[:, 0:W, :] += start_transitions[None, :, :]
    trow = transitions.mean(axis=2)
    for b in range(B):
        L = int(lengths[b])
        for j in range(1, W + 1):
            ucorr[b, L - j] += end_transitions[j - 1]
        for m in (L, L + 1):
            if m >= S:
                continue
            for j in range(1, W + 1):
                r = m - j
                if 0 <= r < L:
                    ucorr[b, r] += trow[j - 1]

    # e4m3 ucorr; masked -> -240 (exp -> 0)
    ucm = np.where(mask[..., None] > 0, ucorr, -240.0).astype(np.float32)
    u8 = ucm.astype(ml_dtypes.float8_e4m3)

    em = np.exp(unary - unary.max(-1, keepdims=True))
    q0 = em / em.sum(-1, keepdims=True)
    q0 = (q0 * mask[..., None] * QSCALE).astype(np.float32)

    def to_dev(x, pad):  # [B,S,T] -> [B, 128, NCH, S(+2*pad)]
        t = x.transpose(0, 2, 1).reshape(B, NCH, 128, S)
        t = np.ascontiguousarray(t.transpose(0, 2, 1, 3))
        if pad:
            tp = np.zeros((B, 128, NCH, S + 2 * pad), t.dtype)
            tp[:, :, :, pad:pad + S] = t
            t = tp
        return t

    qs0 = to_dev(q0, W).astype(ml_dtypes.float8_e4m3)
    u8t = to_dev(u8.astype(np.float32), 0).astype(ml_dtypes.float8_e4m3)

    mats = [transitions[0], transitions[1],
            transitions[0].T, transitions[1].T]
    w8 = np.zeros((128, NCH * 5 * NCH, 128), np.float32)
    for c in range(NCH):
        for mi in range(4):
            for kc in range(NCH):
                w8[:, (c * 5 + mi) * NCH + kc, :] = \
                    mats[mi][kc * 128:(kc + 1) * 128,
                             c * 128:(c + 1) * 128] / QSCALE
        w8[:, (c * 5 + 4) * NCH + c, :] = np.eye(128)
    w8 = w8.astype(ml_dtypes.float8_e4m3)

    wz = np.full((128, NCH, 128), 1.0 / 16, ml_dtypes.float8_e5m2)
    wepse = np.zeros((1, NCH, 128 + HALF), ml_dtypes.float8_e5m2)
    wepse[:, :, 0:128] = np.float32(1.0 / 16)
    wepse[:, :, 128:] = np.float32(2.0 ** -14)

    return qs0, u8t, w8, wz, wepse, ucorr, mask


def kernel(token_feats, unary_score, mask, transitions, start_transitions,
           end_transitions, lengths):
    qs0, u8t, w8, wz, wepse, ucorr, maskf = _host_prep(
        token_feats, unary_score, mask, transitions, start_transitions,
        end_transitions, lengths)

    in_maps = []
    for core in range(N_CORES):
        sl = slice(core * BPC, (core + 1) * BPC)
        in_maps.append({
            "qs0": np.ascontiguousarray(qs0[sl]),
            "u8": np.ascontiguousarray(u8t[sl]),
            "w8": w8,
            "wz": wz,
            "wepse": wepse,
        })

    nc = _get_nc()
    res = bass_utils.run_bass_kernel_spmd(nc, in_maps,
                                          core_ids=list(range(N_CORES)))
    qT = np.concatenate(
        [np.asarray(res.results[c]["qout"], dtype=np.float32)
         for c in range(N_CORES)], axis=0)  # [B, 128, NCH, S]
    msg = qT.transpose(0, 3, 2, 1).reshape(B, S, T)
    out = (ucorr + msg) * maskf[..., None]
    return np.ascontiguousarray(out.astype(np.float32))



# revision 29
# speedup vs baseline: 1.0461x; 1.0071x over previous
"""Trainium2 Bass kernel for the MFVI second-order CRF message passing.

fp8-DoubleRow design (vs fp32r shifted-conv baseline):
  * conv matmuls in e4m3 with MatmulPerfMode.DoubleRow: one instruction
    covers the full 256-deep contraction at 0.5 cycles/output-row (4x
    the fp32r rate); +-1/+-2 sequence shifts are free-dim byte offsets
    of the moving operand into a guard-padded [128, 2, S+4] tile.
  * the unary+corrections add is folded INTO each conv PSUM group as
    extra DoubleRow matmuls: identity-block weights x fp8(ucorr)
    moving operand (iter 0: single e4m3; iter 1: e4m3 + e4m3-residual
    pair for near-exact add, since its output feeds the final conv).
  * softmax chain per iteration: Act exp reads msg+ucorr straight from
    PSUM with bias=-ln(512) (e5m2 range guard) and writes e5m2 "t";
    colsum = e5m2 DoubleRow ones/16-matmul + a 1-partition epsilon
    matmul (masked positions give Z=eps, never 0 -> no inf/NaN);
    rb = 1/pz on DVE; conv input qsm8 = t*rb (= 16*qhat) in e4m3,
    chunk0 on DVE, chunk1 on Pool.
  * iteration 0 is host-side input prep: 16*softmax(unary) shipped as
    e4m3, so the device pipeline starts with a conv.
  * masked positions: ucorr = -240 -> exp underflows to exact 0; the
    reference's uniform-softmax messages from masked senders into
    len-1/len-2 are host-folded into the unary corrections.
  * final iteration: raw msg2 PSUM evacuated to bf16 (DVE st0 / Act
    st1), and (ucorr + msg2) * mask runs on the host in f32.
  * software pipeline: per step [conv | mid-conv colsum of previous
    step | exp/prep | zchain tail], one-step lag, deep SBUF buffer
    rotation; PSUM: 2 conv tags x 3 bufs x 1 bank + colsum 2 banks.
Data-parallel over batch: 4 elems per core, 8 cores, no collectives.
"""
import sys

sys.path.insert(0, "/opt/trn_rl_repo")

import math
import numpy as np
import ml_dtypes

import concourse.mybir as mybir
from concourse.bass import Bass
from concourse.tile import TileContext
from concourse import bass_utils

B, S, T = 32, 1024, 256
W = 2
ITERS = 3
N_CORES = 8
BPC = B // N_CORES
NCH = T // 128
HALF = S // 2
SP = S + 2 * W
K_EXP = 512.0
QSCALE = 16.0
LN_K = math.log(K_EXP)

f32 = mybir.dt.float32
bf16 = mybir.dt.bfloat16
e4 = mybir.dt.float8e4
e5 = mybir.dt.float8e5

SHIFT_D = (-1, -2, +1, +2)  # mats: T1, T2, T1^T, T2^T


def _split_sync_waits(nc):
    ctr = 0
    for f in nc.m.functions:
        for block in f.blocks:
            out = []
            changed = False
            for inst in block.instructions:
                si = inst.sync_info
                waits = list(si.on_wait) if si is not None and si.on_wait else []
                if len(waits) > 1:
                    changed = True
                    for w in waits[:-1]:
                        ctr += 1
                        nop = mybir.InstNoOp(
                            name=f"I-waitsplit-{ctr}",
                            engine=inst.engine, ins=[], outs=[])
                        nop.sync_info = mybir.SyncInfo(on_wait=[w], on_update=[])
                        out.append(nop)
                    si.on_wait = [waits[-1]]
                    inst.sync_info = si
                out.append(inst)
            if changed:
                block.instructions = out
    return nc


def _build():
    nc = Bass(trn_type="TRN2", target_bir_lowering=False, debug=False,
              num_devices=N_CORES)

    qs0_d = nc.dram_tensor("qs0", [BPC, 128, NCH, SP], e4,
                           kind="ExternalInput").ap()
    u8_d = nc.dram_tensor("u8", [BPC, 128, NCH, S], e4,
                          kind="ExternalInput").ap()
    w8_d = nc.dram_tensor("w8", [128, NCH * 5 * NCH, 128], e4,
                          kind="ExternalInput").ap()
    wz_d = nc.dram_tensor("wz", [128, NCH, 128], e5,
                          kind="ExternalInput").ap()
    wepse_d = nc.dram_tensor("wepse", [1, NCH, 128 + HALF], e5,
                             kind="ExternalInput").ap()
    qout = nc.dram_tensor("qout", [BPC, 128, NCH, S], bf16,
                          kind="ExternalOutput").ap()

    DR = mybir.MatmulPerfMode.DoubleRow

    with TileContext(nc) as tc:
        with tc.tile_pool(name="persist", bufs=1) as pp, \
             tc.tile_pool(name="work", bufs=2) as wp, \
             tc.tile_pool(name="psum", bufs=2, space="PSUM") as psp:

            qsA = [pp.tile([128, NCH, SP], e4, tag=f"qsa{n}", name=f"qsa{n}")
                   for n in range(BPC)]
            qsB = [pp.tile([128, NCH, SP], e4, tag=f"qsb{n}", name=f"qsb{n}")
                   for n in range(BPC)]
            w8all = pp.tile([128, NCH * 5 * NCH, 128], e4,
                            tag="w8", name="w8all")
            wzt = pp.tile([128, NCH, 128], e5, tag="wz", name="wzt")
            wepse = pp.tile([1, NCH, 128 + HALF], e5, tag="wepse",
                            name="wepse")
            wepst = wepse[:, :, 0:128]
            epst = wepse[:, :, 128:128 + HALF]
            u8t = [pp.tile([128, NCH, S], e4, tag=f"u8{n}", name=f"u8{n}")
                   for n in range(BPC)]

            # ramp-starter tensors first: one tiny matmul sets pe_busy_start
            # so real convs hit full p-state; no long warm-up needed.
            ones_f = pp.tile([128, 16], f32, tag="ones_f", name="ones_f")
            nc.vector.memset(ones_f[:], 1.0)
            ones_m = pp.tile([128, 16], mybir.dt.float32r, tag="ones_m",
                             name="ones_m")
            nc.vector.tensor_copy(out=ones_m[:], in_=ones_f[:])

            # input DMAs ordered for earliest first-conv start; u8[0] split so
            # its first half lands before the it0/n0 identity matmuls.
            nc.sync.dma_start(out=qsA[0], in_=qs0_d[0])
            nc.sync.dma_start(out=w8all, in_=w8_d)
            nc.sync.dma_start(out=u8t[0][:, :, 0:HALF], in_=u8_d[0][:, :, 0:HALF])
            nc.sync.dma_start(out=u8t[0][:, :, HALF:S], in_=u8_d[0][:, :, HALF:S])
            for n in range(1, BPC):
                nc.sync.dma_start(out=qsA[n], in_=qs0_d[n])
                nc.sync.dma_start(out=u8t[n], in_=u8_d[n])
            nc.sync.dma_start(out=wzt, in_=wz_d)
            nc.sync.dma_start(out=wepse, in_=wepse_d)

            bln = pp.tile([128, 1], f32, tag="bln", name="bln")
            nc.vector.memset(bln[:], -LN_K)

            zg = pp.tile([128, NCH, W], f32, tag="zg", name="zg")
            nc.vector.memset(zg[:], 0.0)
            for n in range(BPC):
                nc.vector.tensor_copy(out=qsB[n][:, :, 0:W], in_=zg[:])
                nc.vector.tensor_copy(out=qsB[n][:, :, S + W:SP], in_=zg[:])

            pwarm = psp.tile([128, S], f32, tag="z", name="pwarm", bufs=1)
            nc.tensor.matmul(pwarm[0:16, 0:16], ones_m[:], ones_m[:],
                             start=True, stop=True)

            def emit_conv_st(it, n, qs_ap, st, pmq):
                # one [128, NCH, HALF] PSUM tile per st: both chunk groups
                # accumulate into disjoint regions, so exp/evac can read
                # both chunks with a single instruction. Shifts for both
                # chunks first, then the ucorr adds: the first step's
                # shift inputs land before u8 does.
                pm = psp.tile([128, NCH, HALF], f32, tag="mm",
                              name=f"pm_{it}_{n}_{st}", bufs=3)
                pmq[st] = pm
                for c in range(NCH):
                    for mi, d in enumerate(SHIFT_D):
                        lo = W + st * HALF + d
                        b0 = (c * 5 + mi) * NCH
                        nc.tensor.matmul(
                            pm[:, c], w8all[:, b0:b0 + 2, :],
                            qs_ap[:, :, lo:lo + HALF],
                            start=(mi == 0), stop=False,
                            perf_mode=DR)
                for c in range(NCH):
                    # +ucorr via fp8 identity (single-rounded e4m3 noise
                    # stays within the rel-err budget for all iterations)
                    bi = (c * 5 + 4) * NCH
                    nc.tensor.matmul(
                        pm[:, c], w8all[:, bi:bi + 2, :],
                        u8t[n][:, :, st * HALF:(st + 1) * HALF],
                        start=False, stop=True, perf_mode=DR)
                return pmq

            def emit_conv(it, n, qs_ap):
                pmq = {}
                emit_conv_st(it, n, qs_ap, 0, pmq)
                emit_conv_st(it, n, qs_ap, 1, pmq)
                return pmq

            def emit_conv_it2(n, qs_ap):
                # final-iteration conv: raw msg2 in PSUM; evac emitted
                # separately (after the ripe zchain tail) so critical
                # normalization legs get engine priority.
                pmq = {}
                for st in range(2):
                    pm = psp.tile([128, NCH, HALF], f32, tag="mm",
                                  name=f"pm_2_{n}_{st}", bufs=3)
                    pmq[st] = pm
                    for c in range(NCH):
                        for mi, d in enumerate(SHIFT_D):
                            lo = W + st * HALF + d
                            b0 = (c * 5 + mi) * NCH
                            nc.tensor.matmul(
                                pm[:, c], w8all[:, b0:b0 + 2, :],
                                qs_ap[:, :, lo:lo + HALF],
                                start=(mi == 0), stop=(mi == 3),
                                perf_mode=DR)
                return pmq

            def emit_evac_it2(n, pmq):
                # PSUM -> bf16 SBUF: Act takes whole-st copies (its exps
                # are done by mid-it2); the last batch elem splits across
                # DVE/Pool, whose zchain legs have finished by then.
                evt = wp.tile([128, NCH, S], bf16, tag="ev",
                              name=f"ev_{n}", bufs=3)
                last = n == BPC - 1
                for st in range(2):
                    sl = slice(st * HALF, (st + 1) * HALF)
                    if last:
                        nc.vector.tensor_copy(out=evt[:, 0, sl],
                                              in_=pmq[st][:, 0])
                        nc.gpsimd.tensor_copy(out=evt[:, 1, sl],
                                              in_=pmq[st][:, 1])
                        nc.sync.dma_start(out=qout[n][:, :, sl],
                                          in_=evt[:, :, sl])
                    else:
                        nc.scalar.copy(out=evt[:, :, sl], in_=pmq[st][:])
                if not last:
                    nc.sync.dma_start(out=qout[n], in_=evt)

            def emit_prep(it_next, n, pmq):
                # t = exp(pm - ln512) from PSUM (Act), e5m2; one exp per st
                e1 = wp.tile([128, NCH, S], e5, tag="e1",
                             name=f"e1_{it_next}_{n}", bufs=8)
                for st in range(2):
                    nc.scalar.activation(
                        out=e1[:, :, st * HALF:(st + 1) * HALF],
                        in_=pmq[st][:],
                        func=mybir.ActivationFunctionType.Exp,
                        bias=bln[:])
                return e1, e1

            def emit_zchain_colsum(n, qu):
                pz = psp.tile([128, S], f32, tag="z", name=f"z_{n}", bufs=1)
                for h in range(2):
                    nc.tensor.matmul(pz[:, h * HALF:(h + 1) * HALF],
                                     wepst[:], epst[:],
                                     start=True, stop=False, perf_mode=DR)
                    nc.tensor.matmul(pz[:, h * HALF:(h + 1) * HALF],
                                     wzt[:], qu[:, :, h * HALF:(h + 1) * HALF],
                                     start=False, stop=True, perf_mode=DR)
                return pz

            SPL = HALF + 2  # conv-st0 reads qs cols [0,516); split past it

            def emit_zchain_tail(n, e1, pz, qs_write):
                # split reciprocal per half so the consumer conv's st0 legs
                # start ~600ns earlier; per half: Pool takes most of c1,
                # DVE takes c0 plus the head of c1 so both legs finish
                # together (DVE 1.04 ns/elem vs Pool 2.16).
                rb = wp.tile([128, S], f32, tag="rb", name=f"rb_{n}", bufs=8)
                nc.vector.reciprocal(rb[:, 0:SPL], pz[:, 0:SPL])
                nc.gpsimd.tensor_mul(out=qs_write[:, 1, W:W + SPL],
                                     in0=e1[:, 1, 0:SPL], in1=rb[:, 0:SPL])
                nc.vector.tensor_mul(out=qs_write[:, 0, W:W + SPL],
                                     in0=e1[:, 0, 0:SPL], in1=rb[:, 0:SPL])
                nc.vector.reciprocal(rb[:, SPL:S], pz[:, SPL:S])
                nc.gpsimd.tensor_mul(out=qs_write[:, 1, W + SPL:W + S],
                                     in0=e1[:, 1, SPL:S], in1=rb[:, SPL:S])
                nc.vector.tensor_mul(out=qs_write[:, 0, W + SPL:W + S],
                                     in0=e1[:, 0, SPL:S], in1=rb[:, SPL:S])

            STEPS = [(0, 0), (0, 1), (0, 2), (0, 3),
                     (1, 0), (1, 1), (1, 2), (1, 3),
                     (2, 0), (2, 1), (2, 2), (2, 3)]
            pending = []
            for it, n in STEPS:
                qs_read = qsA[n] if it != 1 else qsB[n]
                ripe = pending[0] if pending and pending[0][0] >= 1 else None
                if it < 2:
                    pmq = {}
                    emit_conv_st(it, n, qs_read, 0, pmq)
                    if ripe:
                        zn, e1p, qup, qsw = ripe[1]
                        ripe.append(emit_zchain_colsum(zn, qup))
                    emit_conv_st(it, n, qs_read, 1, pmq)
                    if ripe:
                        pending.pop(0)
                        zn, e1p, qup, qsw = ripe[1]
                        emit_zchain_tail(zn, e1p, ripe[2], qsw)
                        ripe = None
                    qs_write = qsB[n] if it == 0 else qsA[n]
                    e1, qu = emit_prep(it + 1, n, pmq)
                    pending.append([0, (n, e1, qu, qs_write)])
                else:
                    pmq = emit_conv_it2(n, qs_read)
                    if ripe:
                        zn, e1p, qup, qsw = ripe[1]
                        ripe.append(emit_zchain_colsum(zn, qup))
                    if ripe:
                        pending.pop(0)
                        zn, e1p, qup, qsw = ripe[1]
                        emit_zchain_tail(zn, e1p, ripe[2], qsw)
                    emit_evac_it2(n, pmq)
                for p in pending:
                    p[0] += 1
            while pending:
                p = pending.pop(0)
                pz = emit_zchain_colsum(p[1][0], p[1][2])
                emit_zchain_tail(p[1][0], p[1][1], pz, p[1][3])

    _split_sync_waits(nc)
    return nc


_CACHED_NC = None


def _get_nc():
    global _CACHED_NC
    if _CACHED_NC is None:
        _CACHED_NC = _build()
    return _CACHED_NC


def _host_prep(token_feats, unary_score, mask, transitions, start_transitions,
               end_transitions, lengths):
    mask = np.asarray(mask, np.float32)
    unary_score = np.asarray(unary_score, np.float32)
    transitions = np.asarray(transitions, np.float32)
    start_transitions = np.asarray(start_transitions, np.float32)
    end_transitions = np.asarray(end_transitions, np.float32)
    lengths = np.asarray(lengths).astype(np.int64)

    unary = unary_score * mask[..., None]

    ucorr = unary.copy()
    ucorr[:, 0:W, :] += start_transitions[None, :, :]
    trow = transitions.mean(axis=2)
    for b in range(B):
        L = int(lengths[b])
        for j in range(1, W + 1):
            ucorr[b, L - j] += end_transitions[j - 1]
        for m in (L, L + 1):
            if m >= S:
                continue
            for j in range(1, W + 1):
                r = m - j
                if 0 <= r < L:
                    ucorr[b, r] += trow[j - 1]

    # e4m3 ucorr; masked -> -240 (exp -> 0)
    ucm = np.where(mask[..., None] > 0, ucorr, -240.0).astype(np.float32)
    u8 = ucm.astype(ml_dtypes.float8_e4m3)

    em = np.exp(unary - unary.max(-1, keepdims=True))
    q0 = em / em.sum(-1, keepdims=True)
    q0 = (q0 * mask[..., None] * QSCALE).astype(np.float32)

    def to_dev(x, pad):  # [B,S,T] -> [B, 128, NCH, S(+2*pad)]
        t = x.transpose(0, 2, 1).reshape(B, NCH, 128, S)
        t = np.ascontiguousarray(t.transpose(0, 2, 1, 3))
        if pad:
            tp = np.zeros((B, 128, NCH, S + 2 * pad), t.dtype)
            tp[:, :, :, pad:pad + S] = t
            t = tp
        return t

    qs0 = to_dev(q0, W).astype(ml_dtypes.float8_e4m3)
    u8t = to_dev(u8.astype(np.float32), 0).astype(ml_dtypes.float8_e4m3)

    mats = [transitions[0], transitions[1],
            transitions[0].T, transitions[1].T]
    w8 = np.zeros((128, NCH * 5 * NCH, 128), np.float32)
    for c in range(NCH):
        for mi in range(4):
            for kc in range(NCH):
                w8[:, (c * 5 + mi) * NCH + kc, :] = \
                    mats[mi][kc * 128:(kc + 1) * 128,
                             c * 128:(c + 1) * 128] / QSCALE
        w8[:, (c * 5 + 4) * NCH + c, :] = np.eye(128)
    w8 = w8.astype(ml_dtypes.float8_e4m3)

    wz = np.full((128, NCH, 128), 1.0 / 16, ml_dtypes.float8_e5m2)
    wepse = np.zeros((1, NCH, 128 + HALF), ml_dtypes.float8_e5m2)
    wepse[:, :, 0:128] = np.float32(1.0 / 16)
    wepse[:, :, 128:] = np.float32(2.0 ** -14)

    return qs0, u8t, w8, wz, wepse, ucorr, mask


def kernel(token_feats, unary_score, mask, transitions, start_transitions,
           end_transitions, lengths):
    qs0, u8t, w8, wz, wepse, ucorr, maskf = _host_prep(
        token_feats, unary_score, mask, transitions, start_transitions,
        end_transitions, lengths)

    in_maps = []
    for core in range(N_CORES):
        sl = slice(core * BPC, (core + 1) * BPC)
        in_maps.append({
            "qs0": np.ascontiguousarray(qs0[sl]),
            "u8": np.ascontiguousarray(u8t[sl]),
            "w8": w8,
            "wz": wz,
            "wepse": wepse,
        })

    nc = _get_nc()
    res = bass_utils.run_bass_kernel_spmd(nc, in_maps,
                                          core_ids=list(range(N_CORES)))
    qT = np.concatenate(
        [np.asarray(res.results[c]["qout"], dtype=np.float32)
         for c in range(N_CORES)], axis=0)  # [B, 128, NCH, S]
    msg = qT.transpose(0, 3, 2, 1).reshape(B, S, T)
    out = (ucorr + msg) * maskf[..., None]
    return np.ascontiguousarray(out.astype(np.float32))



# revision 30
# speedup vs baseline: 1.1531x; 1.1023x over previous
"""Trainium2 Bass kernel for the MFVI second-order CRF message passing.

fp8-DoubleRow design (vs fp32r shifted-conv baseline):
  * conv matmuls in e4m3 with MatmulPerfMode.DoubleRow: one instruction
    covers the full 256-deep contraction at 0.5 cycles/output-row (4x
    the fp32r rate); +-1/+-2 sequence shifts are free-dim byte offsets
    of the moving operand into a guard-padded [128, 2, S+4] tile.
  * the unary+corrections add is folded INTO each conv PSUM group as
    extra DoubleRow matmuls: identity-block weights x fp8(ucorr)
    moving operand (iter 0: single e4m3; iter 1: e4m3 + e4m3-residual
    pair for near-exact add, since its output feeds the final conv).
  * softmax chain per iteration: Act exp reads msg+ucorr straight from
    PSUM with bias=-ln(512) (e5m2 range guard) and writes e5m2 "t";
    colsum = e5m2 DoubleRow ones/16-matmul + a 1-partition epsilon
    matmul (masked positions give Z=eps, never 0 -> no inf/NaN);
    rb = 1/pz on DVE; conv input qsm8 = t*rb (= 16*qhat) in e4m3,
    chunk0 on DVE, chunk1 on Pool.
  * iteration 0 is host-side input prep: 16*softmax(unary) shipped as
    e4m3, so the device pipeline starts with a conv.
  * masked positions: ucorr = -240 -> exp underflows to exact 0; the
    reference's uniform-softmax messages from masked senders into
    len-1/len-2 are host-folded into the unary corrections.
  * final iteration: raw msg2 PSUM evacuated to bf16 (DVE st0 / Act
    st1), and (ucorr + msg2) * mask runs on the host in f32.
  * software pipeline: per step [conv | mid-conv colsum of previous
    step | exp/prep | zchain tail], one-step lag, deep SBUF buffer
    rotation; PSUM: 2 conv tags x 3 bufs x 1 bank + colsum 2 banks.
Data-parallel over batch: 4 elems per core, 8 cores, no collectives.
"""
import sys

sys.path.insert(0, "/opt/trn_rl_repo")

import math
import numpy as np
import ml_dtypes

import concourse.mybir as mybir
from concourse.bass import Bass
from concourse.tile import TileContext
from concourse import bass_utils

B, S, T = 32, 1024, 256
W = 2
ITERS = 3
N_CORES = 8
BPC = B // N_CORES
NCH = T // 128
HALF = S // 2
SP = S + 2 * W
K_EXP = 512.0
QSCALE = 16.0
LN_K = math.log(K_EXP)

f32 = mybir.dt.float32
bf16 = mybir.dt.bfloat16
e4 = mybir.dt.float8e4
e5 = mybir.dt.float8e5

SHIFT_D = (-1, -2, +1, +2)  # mats: T1, T2, T1^T, T2^T


def _split_sync_waits(nc):
    ctr = 0
    for f in nc.m.functions:
        for block in f.blocks:
            out = []
            changed = False
            for inst in block.instructions:
                si = inst.sync_info
                waits = list(si.on_wait) if si is not None and si.on_wait else []
                if len(waits) > 1:
                    changed = True
                    for w in waits[:-1]:
                        ctr += 1
                        nop = mybir.InstNoOp(
                            name=f"I-waitsplit-{ctr}",
                            engine=inst.engine, ins=[], outs=[])
                        nop.sync_info = mybir.SyncInfo(on_wait=[w], on_update=[])
                        out.append(nop)
                    si.on_wait = [waits[-1]]
                    inst.sync_info = si
                out.append(inst)
            if changed:
                block.instructions = out
    return nc


def _build():
    nc = Bass(trn_type="TRN2", target_bir_lowering=False, debug=False,
              num_devices=N_CORES)

    qs0_d = nc.dram_tensor("qs0", [BPC, 128, NCH, SP], e4,
                           kind="ExternalInput").ap()
    u8_d = nc.dram_tensor("u8", [BPC, 128, NCH, S], e4,
                          kind="ExternalInput").ap()
    w8_d = nc.dram_tensor("w8", [128, NCH * 5 * NCH, 128], e4,
                          kind="ExternalInput").ap()
    wz_d = nc.dram_tensor("wz", [128, NCH, 128], e5,
                          kind="ExternalInput").ap()
    wepse_d = nc.dram_tensor("wepse", [1, NCH, 128 + HALF], e5,
                             kind="ExternalInput").ap()
    qout = nc.dram_tensor("qout", [BPC, 128, NCH, S], bf16,
                          kind="ExternalOutput").ap()

    DR = mybir.MatmulPerfMode.DoubleRow

    with TileContext(nc) as tc:
        with tc.tile_pool(name="persist", bufs=1) as pp, \
             tc.tile_pool(name="work", bufs=2) as wp, \
             tc.tile_pool(name="psum", bufs=2, space="PSUM") as psp:

            qsA = [pp.tile([128, NCH, SP], e4, tag=f"qsa{n}", name=f"qsa{n}")
                   for n in range(BPC)]
            qsB = [pp.tile([128, NCH, SP], e4, tag=f"qsb{n}", name=f"qsb{n}")
                   for n in range(BPC)]
            w8all = pp.tile([128, NCH * 5 * NCH, 128], e4,
                            tag="w8", name="w8all")
            wzt = pp.tile([128, NCH, 128], e5, tag="wz", name="wzt")
            wepse = pp.tile([1, NCH, 128 + HALF], e5, tag="wepse",
                            name="wepse")
            wepst = wepse[:, :, 0:128]
            epst = wepse[:, :, 128:128 + HALF]
            u8t = [pp.tile([128, NCH, S], e4, tag=f"u8{n}", name=f"u8{n}")
                   for n in range(BPC)]

            # ramp-starter tensors first: one tiny matmul sets pe_busy_start
            # so real convs hit full p-state; no long warm-up needed.
            ones_f = pp.tile([128, 16], f32, tag="ones_f", name="ones_f")
            nc.vector.memset(ones_f[:], 1.0)
            ones_m = pp.tile([128, 16], mybir.dt.float32r, tag="ones_m",
                             name="ones_m")
            nc.vector.tensor_copy(out=ones_m[:], in_=ones_f[:])

            # input DMAs ordered for earliest first-conv start; u8[0] split so
            # its first half lands before the it0/n0 identity matmuls.
            nc.sync.dma_start(out=qsA[0], in_=qs0_d[0])
            nc.sync.dma_start(out=w8all, in_=w8_d)
            nc.sync.dma_start(out=u8t[0][:, :, 0:HALF], in_=u8_d[0][:, :, 0:HALF])
            nc.sync.dma_start(out=u8t[0][:, :, HALF:S], in_=u8_d[0][:, :, HALF:S])
            for n in range(1, BPC):
                nc.sync.dma_start(out=qsA[n], in_=qs0_d[n])
                nc.sync.dma_start(out=u8t[n], in_=u8_d[n])
            nc.sync.dma_start(out=wzt, in_=wz_d)
            nc.sync.dma_start(out=wepse, in_=wepse_d)

            bln = pp.tile([128, 1], f32, tag="bln", name="bln")
            nc.vector.memset(bln[:], -LN_K)

            zg = pp.tile([128, NCH, W], f32, tag="zg", name="zg")
            nc.vector.memset(zg[:], 0.0)
            for n in range(BPC):
                nc.vector.tensor_copy(out=qsB[n][:, :, 0:W], in_=zg[:])
                nc.vector.tensor_copy(out=qsB[n][:, :, S + W:SP], in_=zg[:])

            pwarm = psp.tile([128, S], f32, tag="z", name="pwarm", bufs=1)
            nc.tensor.matmul(pwarm[0:16, 0:16], ones_m[:], ones_m[:],
                             start=True, stop=True)

            def emit_conv_st(it, n, qs_ap, st, pmq):
                # one [128, NCH, HALF] PSUM tile per st: both chunk groups
                # accumulate into disjoint regions, so exp/evac can read
                # both chunks with a single instruction. Shifts for both
                # chunks first, then the ucorr adds: the first step's
                # shift inputs land before u8 does.
                pm = psp.tile([128, NCH, HALF], f32, tag="mm",
                              name=f"pm_{it}_{n}_{st}", bufs=3)
                pmq[st] = pm
                for c in range(NCH):
                    for mi, d in enumerate(SHIFT_D):
                        lo = W + st * HALF + d
                        b0 = (c * 5 + mi) * NCH
                        nc.tensor.matmul(
                            pm[:, c], w8all[:, b0:b0 + 2, :],
                            qs_ap[:, :, lo:lo + HALF],
                            start=(mi == 0), stop=False,
                            perf_mode=DR)
                for c in range(NCH):
                    # +ucorr via fp8 identity (single-rounded e4m3 noise
                    # stays within the rel-err budget for all iterations)
                    bi = (c * 5 + 4) * NCH
                    nc.tensor.matmul(
                        pm[:, c], w8all[:, bi:bi + 2, :],
                        u8t[n][:, :, st * HALF:(st + 1) * HALF],
                        start=False, stop=True, perf_mode=DR)
                return pmq

            def emit_conv(it, n, qs_ap):
                pmq = {}
                emit_conv_st(it, n, qs_ap, 0, pmq)
                emit_conv_st(it, n, qs_ap, 1, pmq)
                return pmq

            def emit_conv_it2(n, qs_ap):
                # final-iteration conv: raw msg2 in PSUM; evac emitted
                # separately (after the ripe zchain tail) so critical
                # normalization legs get engine priority.
                pmq = {}
                for st in range(2):
                    pm = psp.tile([128, NCH, HALF], f32, tag="mm",
                                  name=f"pm_2_{n}_{st}", bufs=3)
                    pmq[st] = pm
                    for c in range(NCH):
                        for mi, d in enumerate(SHIFT_D):
                            lo = W + st * HALF + d
                            b0 = (c * 5 + mi) * NCH
                            nc.tensor.matmul(
                                pm[:, c], w8all[:, b0:b0 + 2, :],
                                qs_ap[:, :, lo:lo + HALF],
                                start=(mi == 0), stop=(mi == 3),
                                perf_mode=DR)
                return pmq

            def emit_evac_it2(n, pmq):
                # PSUM -> bf16 SBUF: Act takes whole-st copies (its exps
                # are done by mid-it2); the last batch elem splits across
                # DVE/Pool, whose zchain legs have finished by then.
                evt = wp.tile([128, NCH, S], bf16, tag="ev",
                              name=f"ev_{n}", bufs=3)
                last = n == BPC - 1
                for st in range(2):
                    sl = slice(st * HALF, (st + 1) * HALF)
                    if last:
                        nc.vector.tensor_copy(out=evt[:, 0, sl],
                                              in_=pmq[st][:, 0])
                        nc.gpsimd.tensor_copy(out=evt[:, 1, sl],
                                              in_=pmq[st][:, 1])
                        nc.sync.dma_start(out=qout[n][:, :, sl],
                                          in_=evt[:, :, sl])
                    else:
                        nc.scalar.copy(out=evt[:, :, sl], in_=pmq[st][:])
                if not last:
                    nc.sync.dma_start(out=qout[n], in_=evt)

            def emit_prep(it_next, n, pmq):
                # t = exp(pm - ln512) from PSUM (Act), e5m2; one exp per st
                e1 = wp.tile([128, NCH, S], e5, tag="e1",
                             name=f"e1_{it_next}_{n}", bufs=8)
                for st in range(2):
                    nc.scalar.activation(
                        out=e1[:, :, st * HALF:(st + 1) * HALF],
                        in_=pmq[st][:],
                        func=mybir.ActivationFunctionType.Exp,
                        bias=bln[:])
                return e1, e1

            def emit_zchain_colsum(n, qu):
                pz = psp.tile([128, S], f32, tag="z", name=f"z_{n}", bufs=1)
                for h in range(2):
                    nc.tensor.matmul(pz[:, h * HALF:(h + 1) * HALF],
                                     wepst[:], epst[:],
                                     start=True, stop=False, perf_mode=DR)
                    nc.tensor.matmul(pz[:, h * HALF:(h + 1) * HALF],
                                     wzt[:], qu[:, :, h * HALF:(h + 1) * HALF],
                                     start=False, stop=True, perf_mode=DR)
                return pz

            SPL = HALF + 2  # conv-st0 reads qs cols [0,516); split past it

            def emit_zchain_tail(n, e1, pz, qs_write):
                # split reciprocal per half so the consumer conv's st0 legs
                # start ~600ns earlier; per half: Pool takes most of c1,
                # DVE takes c0 plus the head of c1 so both legs finish
                # together (DVE 1.04 ns/elem vs Pool 2.16).
                # single reciprocal up front (frees the pz WAR for the next
                # unit's colsum early), then both engines' legs in parallel
                rb = wp.tile([128, S], f32, tag="rb", name=f"rb_{n}", bufs=8)
                nc.vector.reciprocal(rb[:], pz[:])
                nc.gpsimd.tensor_mul(out=qs_write[:, 1, W:W + SPL],
                                     in0=e1[:, 1, 0:SPL], in1=rb[:, 0:SPL])
                nc.vector.tensor_mul(out=qs_write[:, 0, W:W + SPL],
                                     in0=e1[:, 0, 0:SPL], in1=rb[:, 0:SPL])
                nc.gpsimd.tensor_mul(out=qs_write[:, 1, W + SPL:W + S],
                                     in0=e1[:, 1, SPL:S], in1=rb[:, SPL:S])
                nc.vector.tensor_mul(out=qs_write[:, 0, W + SPL:W + S],
                                     in0=e1[:, 0, SPL:S], in1=rb[:, SPL:S])

            STEPS = [(0, 0), (0, 1), (0, 2), (0, 3),
                     (1, 0), (1, 1), (1, 2), (1, 3),
                     (2, 0), (2, 1), (2, 2), (2, 3)]
            pending = []
            for it, n in STEPS:
                qs_read = qsA[n] if it != 1 else qsB[n]
                ripe = pending[0] if pending and pending[0][0] >= 1 else None
                if it < 2:
                    pmq = {}
                    emit_conv_st(it, n, qs_read, 0, pmq)
                    if ripe:
                        zn, e1p, qup, qsw = ripe[1]
                        ripe.append(emit_zchain_colsum(zn, qup))
                    emit_conv_st(it, n, qs_read, 1, pmq)
                    if ripe:
                        pending.pop(0)
                        zn, e1p, qup, qsw = ripe[1]
                        emit_zchain_tail(zn, e1p, ripe[2], qsw)
                        ripe = None
                    qs_write = qsB[n] if it == 0 else qsA[n]
                    e1, qu = emit_prep(it + 1, n, pmq)
                    pending.append([0, (n, e1, qu, qs_write)])
                else:
                    pmq = emit_conv_it2(n, qs_read)
                    if ripe:
                        zn, e1p, qup, qsw = ripe[1]
                        ripe.append(emit_zchain_colsum(zn, qup))
                    if ripe:
                        pending.pop(0)
                        zn, e1p, qup, qsw = ripe[1]
                        emit_zchain_tail(zn, e1p, ripe[2], qsw)
                    emit_evac_it2(n, pmq)
                for p in pending:
                    p[0] += 1
            while pending:
                p = pending.pop(0)
                pz = emit_zchain_colsum(p[1][0], p[1][2])
                emit_zchain_tail(p[1][0], p[1][1], pz, p[1][3])

    _split_sync_waits(nc)
    return nc


_CACHED_NC = None


def _get_nc():
    global _CACHED_NC
    if _CACHED_NC is None:
        _CACHED_NC = _build()
    return _CACHED_NC


def _host_prep(token_feats, unary_score, mask, transitions, start_transitions,
               end_transitions, lengths):
    mask = np.asarray(mask, np.float32)
    unary_score = np.asarray(unary_score, np.float32)
    transitions = np.asarray(transitions, np.float32)
    start_transitions = np.asarray(start_transitions, np.float32)
    end_transitions = np.asarray(end_transitions, np.float32)
    lengths = np.asarray(lengths).astype(np.int64)

    unary = unary_score * mask[..., None]

    ucorr = unary.copy()
    ucorr[:, 0:W, :] += start_transitions[None, :, :]
    trow = transitions.mean(axis=2)
    for b in range(B):
        L = int(lengths[b])
        for j in range(1, W + 1):
            ucorr[b, L - j] += end_transitions[j - 1]
        for m in (L, L + 1):
            if m >= S:
                continue
            for j in range(1, W + 1):
                r = m - j
                if 0 <= r < L:
                    ucorr[b, r] += trow[j - 1]

    # e4m3 ucorr; masked -> -240 (exp -> 0)
    ucm = np.where(mask[..., None] > 0, ucorr, -240.0).astype(np.float32)
    u8 = ucm.astype(ml_dtypes.float8_e4m3)

    em = np.exp(unary - unary.max(-1, keepdims=True))
    q0 = em / em.sum(-1, keepdims=True)
    q0 = (q0 * mask[..., None] * QSCALE).astype(np.float32)

    def to_dev(x, pad):  # [B,S,T] -> [B, 128, NCH, S(+2*pad)]
        t = x.transpose(0, 2, 1).reshape(B, NCH, 128, S)
        t = np.ascontiguousarray(t.transpose(0, 2, 1, 3))
        if pad:
            tp = np.zeros((B, 128, NCH, S + 2 * pad), t.dtype)
            tp[:, :, :, pad:pad + S] = t
            t = tp
        return t

    qs0 = to_dev(q0, W).astype(ml_dtypes.float8_e4m3)
    u8t = to_dev(u8.astype(np.float32), 0).astype(ml_dtypes.float8_e4m3)

    mats = [transitions[0], transitions[1],
            transitions[0].T, transitions[1].T]
    w8 = np.zeros((128, NCH * 5 * NCH, 128), np.float32)
    for c in range(NCH):
        for mi in range(4):
            for kc in range(NCH):
                w8[:, (c * 5 + mi) * NCH + kc, :] = \
                    mats[mi][kc * 128:(kc + 1) * 128,
                             c * 128:(c + 1) * 128] / QSCALE
        w8[:, (c * 5 + 4) * NCH + c, :] = np.eye(128)
    w8 = w8.astype(ml_dtypes.float8_e4m3)

    wz = np.full((128, NCH, 128), 1.0 / 16, ml_dtypes.float8_e5m2)
    wepse = np.zeros((1, NCH, 128 + HALF), ml_dtypes.float8_e5m2)
    wepse[:, :, 0:128] = np.float32(1.0 / 16)
    wepse[:, :, 128:] = np.float32(2.0 ** -14)

    return qs0, u8t, w8, wz, wepse, ucorr, mask


def kernel(token_feats, unary_score, mask, transitions, start_transitions,
           end_transitions, lengths):
    qs0, u8t, w8, wz, wepse, ucorr, maskf = _host_prep(
        token_feats, unary_score, mask, transitions, start_transitions,
        end_transitions, lengths)

    in_maps = []
    for core in range(N_CORES):
        sl = slice(core * BPC, (core + 1) * BPC)
        in_maps.append({
            "qs0": np.ascontiguousarray(qs0[sl]),
            "u8": np.ascontiguousarray(u8t[sl]),
            "w8": w8,
            "wz": wz,
            "wepse": wepse,
        })

    nc = _get_nc()
    res = bass_utils.run_bass_kernel_spmd(nc, in_maps,
                                          core_ids=list(range(N_CORES)))
    qT = np.concatenate(
        [np.asarray(res.results[c]["qout"], dtype=np.float32)
         for c in range(N_CORES)], axis=0)  # [B, 128, NCH, S]
    msg = qT.transpose(0, 3, 2, 1).reshape(B, S, T)
    out = (ucorr + msg) * maskf[..., None]
    return np.ascontiguousarray(out.astype(np.float32))

